# revision 2
# baseline (speedup 1.0000x reference)
"""Trainium2 Bass kernel for nn_Decoder_10110353014984.

Computation (see reference): hard-reset LIF over T=4 steps followed by a
linear head:
    v' = v + (x_t - v)/2 ; spike = (v' >= 1) ; v = (1-spike) * v'
    y  = einsum('tbnd,cd->tbnc', spikes, W) + b

The LIF replicates the reference's exact fp32 rounding order:
    h = (x*1 - v) ; v' = h*0.5 + v ; spike = v' >= 1 ; v = (v' < 1) * v'
(x*1 and h*0.5 are exact, so the rounding sequence matches v + (x-v)/2).
Exactness matters: a single spike flip changes one output row by a full
W column (~0.2 abs, ~0.18 rel) and would blow the error budget.

Sharding: data-parallel over batch B=64 -> 8 per NeuronCore. The host
pre-transposes each shard to xT[T, D, S] (d-major) so LIF spike tiles are
directly the matmul stationary operand, and pre-transposes W to W^T[D, C].

Numerics: spikes {0,1} exact in bf16; W cast to bf16 on host (measured
rel err 1.8e-3 vs fp32 reference); y stored fp16 on device and upcast on
host (combined rel err ~1.9e-3, well under the 2e-2 gate). bf16 matmul
streams 1 cycle/row and enables fast weight load; fp16 output halves the
dominant DMA-out traffic (25.1 -> 12.5 MB per core).

Engine placement: LIF charge/reset on DVE, spike thresholds on the
otherwise-idle GpSimd engine, PSUM->SBUF(fp16) copies on the Scalar
engine grouped 2 sample-chunks x 2 C-halves per instruction (4 PSUM
banks, ping-ponged) to amortize the ~290ns PSUM access overhead.
"""

import sys
import types

sys.path.insert(0, "/opt/trn_rl_repo")

import numpy as np
import ml_dtypes

import concourse.bass as bass
import concourse.mybir as mybir
import concourse.tile as tile
from concourse.vector_clock import ScopedClock
import bass_rust as _br

T, B, N, D, C = 4, 64, 196, 512, 1000
NCORES = 8
BL = B // NCORES          # 8 batches per core
S = BL * N                # 1568 samples per timestep per core
P = 128                   # partition width
DCH = D // P              # 4 contraction tiles
SCH = (S + P - 1) // P    # 13 sample chunks (last has 32 rows)
CHALF = [(0, 500), (500, 500)]  # C split across two PSUM banks
# sample chunks paired per PSUM group: 6 pairs + 1 tail chunk of 32 rows
GROUPS = [(0, 1), (2, 3), (4, 5), (6, 7), (8, 9), (10, 11), (12,)]

F32 = mybir.dt.float32
F16 = mybir.dt.float16
BF16 = mybir.dt.bfloat16
ALU = mybir.AluOpType


def _patch_tile_drain():
    """This walrus build allows at most one sync wait per TPB_CTRL (Drain)
    instruction; Tile's tail drain carries one wait per active processor.
    Split it into a chain of single-wait drains (same-engine program order
    makes the conjunction equivalent)."""
    if getattr(tile.TileContext, "_drain_split_patch", False):
        return

    def _drain_and_barrier(self, tick_clock, wait_clock):
        drain_inst = self.nc.sync.drain()
        wait_clock.add_sem_waits(
            drain_inst.ins, ScopedClock({None: tick_clock.global_clock})
        )
        waits = (
            list(drain_inst.ins.sync_info.on_wait)
            if drain_inst.ins.has_wait()
            else []
        )
        if len(waits) > 1:
            drain_inst.ins.sync_info.on_wait = waits[:1]
            for i in range(1, len(waits)):
                d2 = self.nc.sync.drain()
                d2.ins.sync_info = _br.SyncInfo(on_wait=waits[i : i + 1], on_update=[])
        self.nc.all_engine_barrier()
        assert self.sems is not None
        popped = self.nc._tile_sem_poison_stack.pop()
        assert popped is self._sem_poison
        self.nc.clear_and_free_semaphores(list(self.sems.allocated().values()))
        self.nc.all_engine_barrier()

    tile.TileContext._drain_and_barrier = _drain_and_barrier

    # Same limit applies to every instruction class (Matmult, DMACopy, ...).
    # Before committing the scheduled instruction stream, shed all but one
    # wait per instruction onto standalone same-engine InstEventSemaphore
    # carriers placed immediately before it (engine program order preserves
    # the conjunction).
    _orig_lower = tile.TileContext._lower_ordered_insts

    def _split_lower(self, ordered):
        for bb_name, insts in ordered.items():
            new = []
            for inst in insts:
                si = inst.sync_info
                if si is not None and len(si.on_wait) > 1:
                    waits = list(si.on_wait)
                    for w in waits[:-1]:
                        ev = mybir.InstEventSemaphore(
                            name=self.nc.get_next_instruction_name(), ins=[], outs=[]
                        )
                        ev.engine = inst.engine
                        ev.sync_info = _br.SyncInfo(on_wait=[w], on_update=[])
                        new.append(ev)
                    inst.sync_info = _br.SyncInfo(
                        on_wait=[waits[-1]], on_update=list(si.on_update)
                    )
                new.append(inst)
            ordered[bb_name] = new
        return _orig_lower(self, ordered)

    tile.TileContext._lower_ordered_insts = _split_lower
    tile.TileContext._drain_split_patch = True


def _install_ntff_hook():
    """Register the axon NTFF profile hook missing from this image's antenv,
    so run_bass_kernel_spmd(trace=True) can report HW exec time."""
    if "antenv.axon_hooks" in sys.modules:
        return
    try:
        import antenv
        from trn_agent_boot.trn_boot import _ntff_profile_via_ctypes

        hook = _ntff_profile_via_ctypes("/opt/axon/libaxon_pjrt.so")
        mod = types.ModuleType("antenv.axon_hooks")
        mod.get_axon_ntff_profile_hook = lambda: hook
        mod.set_axon_ntff_profile_hook = lambda h: None
        sys.modules["antenv.axon_hooks"] = mod
        antenv.axon_hooks = mod
    except Exception:
        pass  # tracing degrades; execution still works


def build_nc():
    """One SPMD NeuronCore program; all 8 cores run it on their own shard."""
    _patch_tile_drain()
    nc = bass.Bass()
    xT = nc.dram_tensor("xT", [T, D, S], F32, kind="ExternalInput")
    wT = nc.dram_tensor("wT", [D, C], BF16, kind="ExternalInput")
    y = nc.dram_tensor("y", [T, S, C], F16, kind="ExternalOutput")

    with tile.TileContext(nc) as tc:
        with (
            tc.tile_pool(name="wpool", bufs=1) as wpool,
            tc.tile_pool(name="vpool", bufs=1) as vpool,
            tc.tile_pool(name="xpool", bufs=8) as xpool,
            tc.tile_pool(name="spool", bufs=2) as spool,
            tc.tile_pool(name="opool", bufs=4) as opool,
            tc.tile_pool(name="ppool", bufs=2, space="PSUM") as ppool,
        ):
            # Startup-critical ordering (subtile deps let consumers start on
            # partially-loaded tiles): first column-quarter of x(t=0) loads
            # first, then W (needed by the first matmul), then the rest of x0.
            QS = [(0, 384), (384, 384), (768, 384), (1152, S - 1152)]
            x0 = [xpool.tile([P, S], F32, tag="x", name=f"x0{d}") for d in range(DCH)]
            q0, qn = QS[0]
            for d in range(DCH):
                nc.sync.dma_start(
                    out=x0[d][:, q0 : q0 + qn],
                    in_=xT[0, d * P : (d + 1) * P, q0 : q0 + qn],
                )

            wt = []
            for d in range(DCH):
                w = wpool.tile([P, C], BF16, tag=f"w{d}", name=f"w{d}")
                nc.sync.dma_start(out=w[:], in_=wT[d * P : (d + 1) * P, :])
                wt.append(w)

            for q0, qn in QS[1:]:
                for d in range(DCH):
                    nc.sync.dma_start(
                        out=x0[d][:, q0 : q0 + qn],
                        in_=xT[0, d * P : (d + 1) * P, q0 : q0 + qn],
                    )

            v = [None] * DCH
            xnext = x0
            for t in range(T):
                xcur, xnext = xnext, []
                sp = []
                if t == 0:
                    for d in range(DCH):
                        sp.append(
                            spool.tile([P, S], BF16, tag=f"sp{d}", name=f"sp{d}")
                        )
                        v[d] = vpool.tile([P, S], F32, tag=f"v{d}", name=f"v{d}")
                    for q0, qn in QS:
                        for d in range(DCH):
                            xq = xcur[d][:, q0 : q0 + qn]
                            # v' = 0.5*x (exact; matches v + (x-v)/2 with v=0)
                            nc.vector.tensor_scalar(
                                out=xq, in0=xq, scalar1=0.5, scalar2=None,
                                op0=ALU.mult,
                            )
                            nc.gpsimd.tensor_scalar(
                                out=sp[d][:, q0 : q0 + qn], in0=xq,
                                scalar1=1.0, scalar2=None, op0=ALU.is_ge,
                            )
                            nc.vector.scalar_tensor_tensor(
                                out=v[d][:, q0 : q0 + qn], in0=xq, scalar=1.0,
                                in1=xq, op0=ALU.is_lt, op1=ALU.mult,
                            )
                else:
                    for d in range(DCH):
                        xt = xcur[d]
                        # h = (x*1 - v), then v' = (h * 0.5) + v -- exact
                        # replication of the reference's rounding order
                        nc.vector.scalar_tensor_tensor(
                            out=xt[:], in0=xt[:], scalar=1.0, in1=v[d][:],
                            op0=ALU.mult, op1=ALU.subtract,
                        )
                        nc.vector.scalar_tensor_tensor(
                            out=xt[:], in0=xt[:], scalar=0.5, in1=v[d][:],
                            op0=ALU.mult, op1=ALU.add,
                        )
                        st = spool.tile([P, S], BF16, tag=f"sp{d}", name=f"sp{d}")
                        nc.gpsimd.tensor_scalar(
                            out=st[:], in0=xt[:], scalar1=1.0, scalar2=None,
                            op0=ALU.is_ge,
                        )
                        sp.append(st)
                    if t < T - 1:
                        for d in range(DCH):
                            # v = (v' < 1) * v' (exact reset, spike in {0,1})
                            nc.vector.scalar_tensor_tensor(
                                out=v[d][:], in0=xcur[d][:], scalar=1.0,
                                in1=xcur[d][:], op0=ALU.is_lt, op1=ALU.mult,
                            )

                if t + 1 < T:
                    for d in range(DCH):
                        xt = xpool.tile([P, S], F32, tag="x", name=f"x{t+1}{d}")
                        nc.sync.dma_start(
                            out=xt[:], in_=xT[t + 1, d * P : (d + 1) * P, :]
                        )
                        xnext.append(xt)

                for g, chunks in enumerate(GROUPS):
                    # 2 sample chunks x 2 C-halves per 4-bank PSUM group;
                    # ppool bufs=2 ping-pongs groups through all 8 banks.
                    ps = ppool.tile([P, 4, 512], F32, tag="ps")
                    ot = opool.tile([P, 2 * C], F16, tag="out")
                    for j, k in enumerate(chunks):
                        col0 = k * P
                        m = min(P, S - col0)
                        for ci, (c0, cn) in enumerate(CHALF):
                            for d in range(DCH):
                                nc.tensor.matmul(
                                    ps[:m, 2 * j + ci, :cn],
                                    sp[d][:, col0 : col0 + m],
                                    wt[d][:, c0 : c0 + cn],
                                    start=(d == 0),
                                    stop=(d == DCH - 1),
                                )
                    nseg = 2 * len(chunks)
                    m = min(P, S - chunks[0] * P)
                    nc.scalar.copy(
                        out=ot[:m, : nseg * 500], in_=ps[:m, :nseg, :500]
                    )
                    r0 = chunks[0] * P
                    rn = sum(min(P, S - k * P) for k in chunks)
                    if len(chunks) == 2:
                        dst = y[t, r0 : r0 + rn, :].rearrange(
                            "(j p) c -> p j c", j=2
                        )
                        nc.sync.dma_start(out=dst, in_=ot[:, : 2 * C])
                    else:
                        nc.sync.dma_start(
                            out=y[t, r0 : r0 + rn, :], in_=ot[:rn, :C]
                        )
    return nc


_NC_CACHE = {}


def _get_nc():
    if "nc" not in _NC_CACHE:
        _NC_CACHE["nc"] = build_nc()
    return _NC_CACHE["nc"]


def _make_in_maps(x, W):
    WT = np.ascontiguousarray(W.T).astype(ml_dtypes.bfloat16)  # [D, C]
    in_maps = []
    for c in range(NCORES):
        xc = x[:, c * BL : (c + 1) * BL].reshape(T, S, D)
        in_maps.append(
            {"xT": np.ascontiguousarray(xc.transpose(0, 2, 1)), "wT": WT}
        )
    return in_maps


def kernel(x, W, b):
    from concourse.bass_utils import run_bass_kernel_spmd

    _install_ntff_hook()
    x = np.asarray(x, dtype=np.float32)
    W = np.asarray(W, dtype=np.float32)
    b = np.asarray(b, dtype=np.float32)

    nc = _get_nc()
    in_maps = _make_in_maps(x, W)
    res = run_bass_kernel_spmd(nc, in_maps, list(range(NCORES)))
    y = np.concatenate(
        [
            res.results[c]["y"].astype(np.float32).reshape(T, BL, N, C)
            for c in range(NCORES)
        ],
        axis=1,
    )
    if np.any(b):
        y = y + b[None, None, None, :]
    return np.ascontiguousarray(y, dtype=np.float32)


# revision 5
# speedup vs baseline: 3.8360x; 3.8360x over previous
"""Trainium2 Bass kernel for nn_Decoder_10110353014984.

Computation (see reference): hard-reset LIF over T=4 steps followed by a
linear head:
    v' = v + (x_t - v)/2 ; spike = (v' >= 1) ; v = (1-spike) * v'
    y  = einsum('tbnd,cd->tbnc', spikes, W) + b

The LIF replicates the reference's exact fp32 rounding order:
    h = (x*1 - v) ; v' = h*0.5 + v ; spike = v' >= 1 ; v = (v' < 1) * v'
(x*1 and h*0.5 are exact, so the rounding sequence matches v + (x-v)/2).
Exactness matters: a single spike flip changes one output row by a full
W column (~0.2 abs, ~0.18 rel) and would blow the error budget.

Sharding: data-parallel over batch B=64 -> 8 per NeuronCore. The host
pre-transposes each shard to xT[T, D, S] (d-major) so LIF spike tiles are
directly the matmul stationary operand, and pre-transposes W to W^T[D, C].

Numerics: spikes {0,1} exact in bf16; W cast to bf16 on host (measured
rel err 1.8e-3 vs fp32 reference); y stored fp16 on device and upcast on
host (combined rel err ~1.9e-3, well under the 2e-2 gate). bf16 matmul
streams 1 cycle/row and enables fast weight load; fp16 output halves the
dominant DMA-out traffic (25.1 -> 12.5 MB per core).

Engine placement: all LIF two-tensor ops on DVE (GpSimd's software
tensor ops measured ~10x slower AND degrade DVE via SBUF contention);
the single-tensor t=0 charge (0.5*x) on the Scalar engine; PSUM->
SBUF(fp16) copies on the Scalar engine grouped 2 sample-chunks x 2
C-halves per instruction (4 PSUM banks, ping-ponged) to amortize the
~290ns PSUM access overhead. LIF for t>=1 is emitted in column halves
so the first spike tiles land early and the tensor engine never
starves at a timestep boundary. The four 32-row tail sample-chunks
(S = 12*128 + 32) are packed across t into one full 128-row matmul
chunk at the end.
"""

import sys
import types

sys.path.insert(0, "/opt/trn_rl_repo")

import numpy as np
import ml_dtypes

import concourse.bass as bass
import concourse.mybir as mybir
import concourse.tile as tile
from concourse.vector_clock import ScopedClock
import bass_rust as _br

T, B, N, D, C = 4, 64, 196, 512, 1000
NCORES = 8
BL = B // NCORES          # 8 batches per core
S = BL * N                # 1568 samples per timestep per core
P = 128                   # partition width
DCH = D // P              # 4 contraction tiles
SCH = (S + P - 1) // P    # 13 sample chunks (last has 32 rows)
CHALF = [(0, 500), (500, 500)]  # C split across two PSUM banks
# sample chunks paired per PSUM group; the 32-row tail (chunk 12) is
# instead packed across the 4 timesteps into one 128-row chunk
GROUPS = [(0, 1), (2, 3), (4, 5), (6, 7), (8, 9), (10, 11)]
SMAIN = 12 * P            # 1536 samples in the paired groups
STAIL = S - SMAIN         # 32 tail samples per timestep

F32 = mybir.dt.float32
F16 = mybir.dt.float16
BF16 = mybir.dt.bfloat16
ALU = mybir.AluOpType


def _patch_tile_drain():
    """This walrus build allows at most one sync wait per TPB_CTRL (Drain)
    instruction; Tile's tail drain carries one wait per active processor.
    Split it into a chain of single-wait drains (same-engine program order
    makes the conjunction equivalent)."""
    if getattr(tile.TileContext, "_drain_split_patch", False):
        return

    def _drain_and_barrier(self, tick_clock, wait_clock):
        drain_inst = self.nc.sync.drain()
        wait_clock.add_sem_waits(
            drain_inst.ins, ScopedClock({None: tick_clock.global_clock})
        )
        waits = (
            list(drain_inst.ins.sync_info.on_wait)
            if drain_inst.ins.has_wait()
            else []
        )
        if len(waits) > 1:
            drain_inst.ins.sync_info.on_wait = waits[:1]
            for i in range(1, len(waits)):
                d2 = self.nc.sync.drain()
                d2.ins.sync_info = _br.SyncInfo(on_wait=waits[i : i + 1], on_update=[])
        self.nc.all_engine_barrier()
        assert self.sems is not None
        popped = self.nc._tile_sem_poison_stack.pop()
        assert popped is self._sem_poison
        self.nc.clear_and_free_semaphores(list(self.sems.allocated().values()))
        self.nc.all_engine_barrier()

    tile.TileContext._drain_and_barrier = _drain_and_barrier

    # Same limit applies to every instruction class (Matmult, DMACopy, ...).
    # Before committing the scheduled instruction stream, shed all but one
    # wait per instruction onto standalone same-engine InstEventSemaphore
    # carriers placed immediately before it (engine program order preserves
    # the conjunction).
    _orig_lower = tile.TileContext._lower_ordered_insts

    def _split_lower(self, ordered):
        for bb_name, insts in ordered.items():
            new = []
            for inst in insts:
                si = inst.sync_info
                if si is not None and len(si.on_wait) > 1:
                    waits = list(si.on_wait)
                    for w in waits[:-1]:
                        ev = mybir.InstEventSemaphore(
                            name=self.nc.get_next_instruction_name(), ins=[], outs=[]
                        )
                        ev.engine = inst.engine
                        ev.sync_info = _br.SyncInfo(on_wait=[w], on_update=[])
                        new.append(ev)
                    inst.sync_info = _br.SyncInfo(
                        on_wait=[waits[-1]], on_update=list(si.on_update)
                    )
                new.append(inst)
            ordered[bb_name] = new
        return _orig_lower(self, ordered)

    tile.TileContext._lower_ordered_insts = _split_lower
    tile.TileContext._drain_split_patch = True


def _install_ntff_hook():
    """Register the axon NTFF profile hook missing from this image's antenv,
    so run_bass_kernel_spmd(trace=True) can report HW exec time."""
    if "antenv.axon_hooks" in sys.modules:
        return
    try:
        import antenv
        from trn_agent_boot.trn_boot import _ntff_profile_via_ctypes

        hook = _ntff_profile_via_ctypes("/opt/axon/libaxon_pjrt.so")
        mod = types.ModuleType("antenv.axon_hooks")
        mod.get_axon_ntff_profile_hook = lambda: hook
        mod.set_axon_ntff_profile_hook = lambda h: None
        sys.modules["antenv.axon_hooks"] = mod
        antenv.axon_hooks = mod
    except Exception:
        pass  # tracing degrades; execution still works


def build_nc():
    """One SPMD NeuronCore program; all 8 cores run it on their own shard."""
    _patch_tile_drain()
    nc = bass.Bass()
    xT = nc.dram_tensor("xT", [T, D, S], F32, kind="ExternalInput")
    wT = nc.dram_tensor("wT", [D, C], BF16, kind="ExternalInput")
    y = nc.dram_tensor("y", [T, S, C], F16, kind="ExternalOutput")

    with tile.TileContext(nc) as tc:
        with (
            tc.tile_pool(name="wpool", bufs=1) as wpool,
            tc.tile_pool(name="vpool", bufs=1) as vpool,
            tc.tile_pool(name="xpool", bufs=8) as xpool,
            tc.tile_pool(name="spool", bufs=2) as spool,
            tc.tile_pool(name="opool", bufs=4) as opool,
            tc.tile_pool(name="ppool", bufs=2, space="PSUM") as ppool,
        ):
            # Startup-critical ordering (subtile deps let consumers start on
            # partially-loaded tiles): first column-quarter of x(t=0) loads
            # first, then W (needed by the first matmul), then the rest of x0.
            QS = [(0, 384), (384, 384), (768, 384), (1152, S - 1152)]
            x0 = [xpool.tile([P, S], F32, tag="x", name=f"x0{d}") for d in range(DCH)]
            q0, qn = QS[0]
            for d in range(DCH):
                nc.sync.dma_start(
                    out=x0[d][:, q0 : q0 + qn],
                    in_=xT[0, d * P : (d + 1) * P, q0 : q0 + qn],
                )

            wt = []
            for d in range(DCH):
                w = wpool.tile([P, C], BF16, tag=f"w{d}", name=f"w{d}")
                nc.sync.dma_start(out=w[:], in_=wT[d * P : (d + 1) * P, :])
                wt.append(w)

            for q0, qn in QS[1:]:
                for d in range(DCH):
                    nc.sync.dma_start(
                        out=x0[d][:, q0 : q0 + qn],
                        in_=xT[0, d * P : (d + 1) * P, q0 : q0 + qn],
                    )

            def emit_spikes(t, d, xt, sp, tp, q0, qn):
                """spike = v' >= 1 into the main sp tile, with the 32-col
                tail routed into the packed cross-t tail tile instead."""
                lo, hi = q0, q0 + qn
                if lo < SMAIN:
                    mh = min(hi, SMAIN)
                    nc.vector.tensor_scalar(
                        out=sp[d][:, lo:mh], in0=xt[:, lo:mh],
                        scalar1=1.0, scalar2=None, op0=ALU.is_ge,
                    )
                if hi > SMAIN:
                    nc.vector.tensor_scalar(
                        out=tp[d][:, t * STAIL : (t + 1) * STAIL],
                        in0=xt[:, SMAIN:hi],
                        scalar1=1.0, scalar2=None, op0=ALU.is_ge,
                    )

            # packed tail spikes: partition = d-row, col = 32*t + tail sample
            tp = [
                vpool.tile([P, T * STAIL], BF16, tag=f"tp{d}", name=f"tp{d}")
                for d in range(DCH)
            ]
            v = [None] * DCH
            HALVES = [(0, 768), (768, S - 768)]
            xnext = x0
            for t in range(T):
                xcur, xnext = xnext, []
                sp = []
                if t == 0:
                    for d in range(DCH):
                        sp.append(
                            spool.tile([P, SMAIN], BF16, tag=f"sp{d}", name=f"sp{d}")
                        )
                        v[d] = vpool.tile([P, S], F32, tag=f"v{d}", name=f"v{d}")
                    for q0, qn in QS:
                        for d in range(DCH):
                            xq = xcur[d][:, q0 : q0 + qn]
                            # v' = 0.5*x (exact; matches v + (x-v)/2 with
                            # v=0). Single-tensor op -> Scalar engine.
                            nc.scalar.activation(
                                out=xq, in_=xq,
                                func=mybir.ActivationFunctionType.Copy,
                                scale=0.5,
                            )
                            emit_spikes(t, d, xcur[d], sp, tp, q0, qn)
                            nc.vector.scalar_tensor_tensor(
                                out=v[d][:, q0 : q0 + qn], in0=xq, scalar=1.0,
                                in1=xq, op0=ALU.is_lt, op1=ALU.mult,
                            )
                else:
                    for d in range(DCH):
                        sp.append(
                            spool.tile([P, SMAIN], BF16, tag=f"sp{d}", name=f"sp{d}")
                        )
                    # column halves: the first half's spikes across all d
                    # land early so the t-boundary matmuls never starve
                    for q0, qn in HALVES:
                        for d in range(DCH):
                            xq = xcur[d][:, q0 : q0 + qn]
                            vq = v[d][:, q0 : q0 + qn]
                            # h = (x*1 - v), then v' = (h * 0.5) + v --
                            # exact replication of the reference rounding
                            nc.vector.scalar_tensor_tensor(
                                out=xq, in0=xq, scalar=1.0, in1=vq,
                                op0=ALU.mult, op1=ALU.subtract,
                            )
                            nc.vector.scalar_tensor_tensor(
                                out=xq, in0=xq, scalar=0.5, in1=vq,
                                op0=ALU.mult, op1=ALU.add,
                            )
                            emit_spikes(t, d, xcur[d], sp, tp, q0, qn)
                    if t < T - 1:
                        for q0, qn in HALVES:
                            for d in range(DCH):
                                xq = xcur[d][:, q0 : q0 + qn]
                                # v = (v' < 1) * v' (exact hard reset)
                                nc.vector.scalar_tensor_tensor(
                                    out=v[d][:, q0 : q0 + qn], in0=xq,
                                    scalar=1.0, in1=xq,
                                    op0=ALU.is_lt, op1=ALU.mult,
                                )

                if t + 1 < T:
                    for d in range(DCH):
                        xt = xpool.tile([P, S], F32, tag="x", name=f"x{t+1}{d}")
                        nc.sync.dma_start(
                            out=xt[:], in_=xT[t + 1, d * P : (d + 1) * P, :]
                        )
                        xnext.append(xt)

                for g, chunks in enumerate(GROUPS):
                    # 2 sample chunks x 2 C-halves per 4-bank PSUM group;
                    # ppool bufs=2 ping-pongs groups through all 8 banks.
                    ps = ppool.tile([P, 4, 512], F32, tag="ps")
                    ot = opool.tile([P, 2 * C], F16, tag="out")
                    for j, k in enumerate(chunks):
                        col0 = k * P
                        for ci, (c0, cn) in enumerate(CHALF):
                            for d in range(DCH):
                                nc.tensor.matmul(
                                    ps[:P, 2 * j + ci, :cn],
                                    sp[d][:, col0 : col0 + P],
                                    wt[d][:, c0 : c0 + cn],
                                    start=(d == 0),
                                    stop=(d == DCH - 1),
                                )
                    nc.scalar.copy(out=ot[:, : 4 * 500], in_=ps[:, :4, :500])
                    r0 = chunks[0] * P
                    dst = y[t, r0 : r0 + 2 * P, :].rearrange(
                        "(j p) c -> p j c", j=2
                    )
                    nc.sync.dma_start(out=dst, in_=ot[:, : 2 * C])

            # packed tail: one 128-row chunk covering the 32-row tails of
            # all 4 timesteps (partition = 32*t + tail sample)
            ps = ppool.tile([P, 4, 512], F32, tag="ps")
            ot = opool.tile([P, 2 * C], F16, tag="out")
            for ci, (c0, cn) in enumerate(CHALF):
                for d in range(DCH):
                    nc.tensor.matmul(
                        ps[:P, ci, :cn],
                        tp[d][:, :],
                        wt[d][:, c0 : c0 + cn],
                        start=(d == 0),
                        stop=(d == DCH - 1),
                    )
            nc.scalar.copy(out=ot[:, :C], in_=ps[:, :2, :500])
            nc.sync.dma_start(out=y[:, SMAIN:S, :], in_=ot[:, :C])
    return nc


_NC_CACHE = {}


def _get_nc():
    if "nc" not in _NC_CACHE:
        _NC_CACHE["nc"] = build_nc()
    return _NC_CACHE["nc"]


def _make_in_maps(x, W):
    WT = np.ascontiguousarray(W.T).astype(ml_dtypes.bfloat16)  # [D, C]
    in_maps = []
    for c in range(NCORES):
        xc = x[:, c * BL : (c + 1) * BL].reshape(T, S, D)
        in_maps.append(
            {"xT": np.ascontiguousarray(xc.transpose(0, 2, 1)), "wT": WT}
        )
    return in_maps


def kernel(x, W, b):
    from concourse.bass_utils import run_bass_kernel_spmd

    _install_ntff_hook()
    x = np.asarray(x, dtype=np.float32)
    W = np.asarray(W, dtype=np.float32)
    b = np.asarray(b, dtype=np.float32)

    nc = _get_nc()
    in_maps = _make_in_maps(x, W)
    res = run_bass_kernel_spmd(nc, in_maps, list(range(NCORES)))
    y = np.concatenate(
        [
            res.results[c]["y"].astype(np.float32).reshape(T, BL, N, C)
            for c in range(NCORES)
        ],
        axis=1,
    )
    if np.any(b):
        y = y + b[None, None, None, :]
    return np.ascontiguousarray(y, dtype=np.float32)


# revision 11
# speedup vs baseline: 3.9284x; 1.0241x over previous
"""Trainium2 Bass kernel for nn_Decoder_10110353014984.

Computation (see reference): hard-reset LIF over T=4 steps followed by a
linear head:
    v' = v + (x_t - v)/2 ; spike = (v' >= 1) ; v = (1-spike) * v'
    y  = einsum('tbnd,cd->tbnc', spikes, W) + b

The LIF replicates the reference's exact fp32 rounding order:
    h = (x*1 - v) ; v' = h*0.5 + v ; spike = v' >= 1 ; v = (v' < 1) * v'
(x*1 and h*0.5 are exact, so the rounding sequence matches v + (x-v)/2).
Exactness matters: a single spike flip changes one output row by a full
W column (~0.2 abs, ~0.18 rel) and would blow the error budget.

Sharding: data-parallel over batch B=64 -> 8 per NeuronCore. The host
pre-transposes each shard to xT[T, D, S] (d-major) so LIF spike tiles are
directly the matmul stationary operand, and pre-transposes W to W^T[D, C].

Numerics: spikes {0,1} exact in bf16; W cast to bf16 on host (measured
rel err 1.8e-3 vs fp32 reference); y stored fp16 on device and upcast on
host (combined rel err ~1.9e-3, well under the 2e-2 gate). bf16 matmul
streams 1 cycle/row and enables fast weight load; fp16 output halves the
dominant DMA-out traffic (25.1 -> 12.5 MB per core).

Engine placement: all LIF two-tensor ops on DVE (GpSimd's software
tensor ops measured ~10x slower AND degrade DVE via SBUF contention);
the single-tensor t=0 charge (0.5*x) on the Scalar engine; PSUM->
SBUF(fp16) copies on the Scalar engine grouped 2 sample-chunks x 2
C-halves per instruction (4 PSUM banks, ping-ponged) to amortize the
~290ns PSUM access overhead. LIF for t>=1 is emitted in column halves
so the first spike tiles land early and the tensor engine never
starves at a timestep boundary. The four 32-row tail sample-chunks
(S = 12*128 + 32) are packed across t into one full 128-row matmul
chunk at the end.
"""

import sys
import types

sys.path.insert(0, "/opt/trn_rl_repo")

import numpy as np
import ml_dtypes

import concourse.bass as bass
import concourse.mybir as mybir
import concourse.tile as tile
from concourse.vector_clock import ScopedClock
import bass_rust as _br

T, B, N, D, C = 4, 64, 196, 512, 1000
NCORES = 8
BL = B // NCORES          # 8 batches per core
S = BL * N                # 1568 samples per timestep per core
P = 128                   # partition width
DCH = D // P              # 4 contraction tiles
SCH = (S + P - 1) // P    # 13 sample chunks (last has 32 rows)
CHALF = [(0, 500), (500, 500)]  # C split across two PSUM banks
# sample chunks paired per PSUM group; the 32-row tail (chunk 12) is
# instead packed across the 4 timesteps into one 128-row chunk
GROUPS = [(0, 1), (2, 3), (4, 5), (6, 7), (8, 9), (10, 11)]
SMAIN = 12 * P            # 1536 samples in the paired groups
STAIL = S - SMAIN         # 32 tail samples per timestep

F32 = mybir.dt.float32
F16 = mybir.dt.float16
BF16 = mybir.dt.bfloat16
ALU = mybir.AluOpType


def _patch_tile_drain():
    """This walrus build allows at most one sync wait per TPB_CTRL (Drain)
    instruction; Tile's tail drain carries one wait per active processor.
    Split it into a chain of single-wait drains (same-engine program order
    makes the conjunction equivalent)."""
    if getattr(tile.TileContext, "_drain_split_patch", False):
        return

    def _drain_and_barrier(self, tick_clock, wait_clock):
        drain_inst = self.nc.sync.drain()
        wait_clock.add_sem_waits(
            drain_inst.ins, ScopedClock({None: tick_clock.global_clock})
        )
        waits = (
            list(drain_inst.ins.sync_info.on_wait)
            if drain_inst.ins.has_wait()
            else []
        )
        if len(waits) > 1:
            drain_inst.ins.sync_info.on_wait = waits[:1]
            for i in range(1, len(waits)):
                d2 = self.nc.sync.drain()
                d2.ins.sync_info = _br.SyncInfo(on_wait=waits[i : i + 1], on_update=[])
        self.nc.all_engine_barrier()
        assert self.sems is not None
        popped = self.nc._tile_sem_poison_stack.pop()
        assert popped is self._sem_poison
        self.nc.clear_and_free_semaphores(list(self.sems.allocated().values()))
        self.nc.all_engine_barrier()

    tile.TileContext._drain_and_barrier = _drain_and_barrier

    # Same limit applies to every instruction class (Matmult, DMACopy, ...).
    # Before committing the scheduled instruction stream, shed all but one
    # wait per instruction onto standalone same-engine InstEventSemaphore
    # carriers placed immediately before it (engine program order preserves
    # the conjunction).
    _orig_lower = tile.TileContext._lower_ordered_insts

    def _split_lower(self, ordered):
        for bb_name, insts in ordered.items():
            new = []
            for inst in insts:
                si = inst.sync_info
                if si is not None and len(si.on_wait) > 1:
                    waits = list(si.on_wait)
                    for w in waits[:-1]:
                        ev = mybir.InstEventSemaphore(
                            name=self.nc.get_next_instruction_name(), ins=[], outs=[]
                        )
                        ev.engine = inst.engine
                        ev.sync_info = _br.SyncInfo(on_wait=[w], on_update=[])
                        new.append(ev)
                    inst.sync_info = _br.SyncInfo(
                        on_wait=[waits[-1]], on_update=list(si.on_update)
                    )
                new.append(inst)
            ordered[bb_name] = new
        return _orig_lower(self, ordered)

    tile.TileContext._lower_ordered_insts = _split_lower
    tile.TileContext._drain_split_patch = True


def _install_ntff_hook():
    """Register the axon NTFF profile hook missing from this image's antenv,
    so run_bass_kernel_spmd(trace=True) can report HW exec time."""
    if "antenv.axon_hooks" in sys.modules:
        return
    try:
        import antenv
        from trn_agent_boot.trn_boot import _ntff_profile_via_ctypes

        hook = _ntff_profile_via_ctypes("/opt/axon/libaxon_pjrt.so")
        mod = types.ModuleType("antenv.axon_hooks")
        mod.get_axon_ntff_profile_hook = lambda: hook
        mod.set_axon_ntff_profile_hook = lambda h: None
        sys.modules["antenv.axon_hooks"] = mod
        antenv.axon_hooks = mod
    except Exception:
        pass  # tracing degrades; execution still works


def build_nc():
    """One SPMD NeuronCore program; all 8 cores run it on their own shard."""
    _patch_tile_drain()
    nc = bass.Bass()
    xT = nc.dram_tensor("xT", [T, D, S], F32, kind="ExternalInput")
    wT = nc.dram_tensor("wT", [D, C], BF16, kind="ExternalInput")
    y = nc.dram_tensor("y", [T, S, C], F16, kind="ExternalOutput")

    with tile.TileContext(nc) as tc:
        with (
            tc.tile_pool(name="wpool", bufs=1) as wpool,
            tc.tile_pool(name="vpool", bufs=1) as vpool,
            tc.tile_pool(name="xpool", bufs=3) as xpool,
            tc.tile_pool(name="spool", bufs=2) as spool,
            tc.tile_pool(name="opool", bufs=4) as opool,
            tc.tile_pool(name="ppool", bufs=2, space="PSUM") as ppool,
        ):
            # The 4 d-chunks are fused into flat 2D tiles so that a single
            # DMA issue loads all of them (the sync engine's ~0.7us per
            # issue dominated startup) and a single DVE/ACT instruction
            # processes all of them. CRITICAL: Tile's subtile dependency
            # tracking only handles contiguous per-partition intervals, so
            # the flat layout is segment-major: segment (q0,qn) occupies
            # cols [4*q0, 4*q0+4*qn), holding d-major [d][s] blocks. Every
            # SBUF access below is a contiguous column interval.
            QS = [(0, 384), (384, 384), (768, 384), (1152, S - 1152)]

            def xslice(xt, q0, qn):
                return xt[:, 4 * q0 : 4 * q0 + 4 * qn]

            def load_x(t, xt, pieces):
                """DMA dram xT[t] into the seg-major flat tile, one issue
                per segment (3D source AP iterates p,d,s to match the flat
                [seg][d][s] layout)."""
                for q0, qn in pieces:
                    nc.sync.dma_start(
                        out=xslice(xt, q0, qn),
                        in_=xT[t, :, q0 : q0 + qn].rearrange(
                            "(d p) s -> p d s", d=DCH
                        ),
                    )

            # startup-critical order: first x0 quarter, then W, then the rest
            x0 = xpool.tile([P, DCH * S], F32, tag="x", name="x0")
            load_x(0, x0, QS[:1])
            # W flat [P, DCH*C]: d-chunk d at cols [d*C, (d+1)*C)
            wt = wpool.tile([P, DCH * C], BF16, tag="w", name="w")
            nc.sync.dma_start(
                out=wt[:], in_=wT[:, :].rearrange("(d p) c -> p d c", d=DCH)
            )
            load_x(0, x0, QS[1:])

            def emit_spikes(t, xt, sp, tp, qi):
                """spike = v' >= 1. Segments 0-2 go to the sp tile whole;
                segment 3 (416 cols) is emitted per-d: 384 cols to sp and
                the 32-col tail into the packed cross-t tail tile (so all
                writes stay contiguous column intervals)."""
                q0, qn = QS[qi]
                if qi < 3:
                    nc.vector.tensor_scalar(
                        out=sp[:, 1536 * qi : 1536 * (qi + 1)],
                        in0=xslice(xt, q0, qn),
                        scalar1=1.0, scalar2=None, op0=ALU.is_ge,
                    )
                    return
                for d in range(DCH):
                    base = 4 * q0 + d * qn
                    nc.vector.tensor_scalar(
                        out=sp[:, 4608 + d * 384 : 4608 + d * 384 + 384],
                        in0=xt[:, base : base + 384],
                        scalar1=1.0, scalar2=None, op0=ALU.is_ge,
                    )
                    nc.vector.tensor_scalar(
                        out=tp[:, d * P + STAIL * t : d * P + STAIL * (t + 1)],
                        in0=xt[:, base + 384 : base + qn],
                        scalar1=1.0, scalar2=None, op0=ALU.is_ge,
                    )

            # packed tail spikes: col = d*128 + 32*t + tail-sample
            tp = vpool.tile([P, DCH * P], BF16, tag="tp", name="tp")
            v = vpool.tile([P, DCH * S], F32, tag="v", name="v")
            xcur = None
            xnext = x0
            for t in range(T):
                xcur, xnext = xnext, None
                # sp flat layout: segment q holds d-major [d][384] blocks at
                # cols [1536*q, 1536*(q+1)); matmul chunk k = 3q+j reads the
                # contiguous interval 1536*q + d*384 + 128*j.
                sp = spool.tile([P, DCH * SMAIN], BF16, tag="sp", name=f"sp{t}")
                for qi, (q0, qn) in enumerate(QS):
                    xq = xslice(xcur, q0, qn)
                    vq = xslice(v, q0, qn)
                    if t == 0:
                        # v' = 0.5*x (exact; matches v + (x-v)/2 with v=0).
                        # Single-tensor op -> Scalar engine.
                        nc.scalar.activation(
                            out=xq, in_=xq,
                            func=mybir.ActivationFunctionType.Copy,
                            scale=0.5,
                        )
                    else:
                        # h = (x*1 - v), then v' = (h * 0.5) + v -- exact
                        # replication of the reference rounding order
                        nc.vector.scalar_tensor_tensor(
                            out=xq, in0=xq, scalar=1.0, in1=vq,
                            op0=ALU.mult, op1=ALU.subtract,
                        )
                        nc.vector.scalar_tensor_tensor(
                            out=xq, in0=xq, scalar=0.5, in1=vq,
                            op0=ALU.mult, op1=ALU.add,
                        )
                    emit_spikes(t, xcur, sp, tp, qi)
                if t < T - 1:
                    for q0, qn in QS:
                        xq = xslice(xcur, q0, qn)
                        # v = (v' < 1) * v' (exact hard reset)
                        nc.vector.scalar_tensor_tensor(
                            out=xslice(v, q0, qn), in0=xq, scalar=1.0,
                            in1=xq, op0=ALU.is_lt, op1=ALU.mult,
                        )
                    xnext = xpool.tile(
                        [P, DCH * S], F32, tag="x", name=f"x{t+1}"
                    )
                    load_x(t + 1, xnext, QS)

                for g, chunks in enumerate(GROUPS):
                    # 2 sample chunks x 2 C-halves per 4-bank PSUM group;
                    # ppool bufs=2 ping-pongs groups through all 8 banks.
                    ps = ppool.tile([P, 4, 512], F32, tag="ps")
                    ot = opool.tile([P, 2 * C], F16, tag="out")
                    for j, k in enumerate(chunks):
                        sp0 = 1536 * (k // 3) + 128 * (k % 3)
                        for ci, (c0, cn) in enumerate(CHALF):
                            for d in range(DCH):
                                nc.tensor.matmul(
                                    ps[:P, 2 * j + ci, :cn],
                                    sp[:, sp0 + d * 384 : sp0 + d * 384 + P],
                                    wt[:, d * C + c0 : d * C + c0 + cn],
                                    start=(d == 0),
                                    stop=(d == DCH - 1),
                                )
                    nc.scalar.copy(out=ot[:, : 4 * 500], in_=ps[:, :4, :500])
                    r0 = chunks[0] * P
                    dst = y[t, r0 : r0 + 2 * P, :].rearrange(
                        "(j p) c -> p j c", j=2
                    )
                    nc.sync.dma_start(out=dst, in_=ot[:, : 2 * C])

            # packed tail: one 128-row chunk covering the 32-row tails of
            # all 4 timesteps (psum partition = 32*t + tail sample)
            ps = ppool.tile([P, 4, 512], F32, tag="ps")
            ot = opool.tile([P, 2 * C], F16, tag="out")
            for ci, (c0, cn) in enumerate(CHALF):
                for d in range(DCH):
                    nc.tensor.matmul(
                        ps[:P, ci, :cn],
                        tp[:, d * P : (d + 1) * P],
                        wt[:, d * C + c0 : d * C + c0 + cn],
                        start=(d == 0),
                        stop=(d == DCH - 1),
                    )
            nc.scalar.copy(out=ot[:, :C], in_=ps[:, :2, :500])
            nc.sync.dma_start(out=y[:, SMAIN:S, :], in_=ot[:, :C])
    return nc


_NC_CACHE = {}


def _get_nc():
    if "nc" not in _NC_CACHE:
        _NC_CACHE["nc"] = build_nc()
    return _NC_CACHE["nc"]


def _make_in_maps(x, W):
    WT = np.ascontiguousarray(W.T).astype(ml_dtypes.bfloat16)  # [D, C]
    in_maps = []
    for c in range(NCORES):
        xc = x[:, c * BL : (c + 1) * BL].reshape(T, S, D)
        in_maps.append(
            {"xT": np.ascontiguousarray(xc.transpose(0, 2, 1)), "wT": WT}
        )
    return in_maps


def kernel(x, W, b):
    from concourse.bass_utils import run_bass_kernel_spmd

    _install_ntff_hook()
    x = np.asarray(x, dtype=np.float32)
    W = np.asarray(W, dtype=np.float32)
    b = np.asarray(b, dtype=np.float32)

    nc = _get_nc()
    in_maps = _make_in_maps(x, W)
    res = run_bass_kernel_spmd(nc, in_maps, list(range(NCORES)))
    y = np.concatenate(
        [
            res.results[c]["y"].astype(np.float32).reshape(T, BL, N, C)
            for c in range(NCORES)
        ],
        axis=1,
    )
    if np.any(b):
        y = y + b[None, None, None, :]
    return np.ascontiguousarray(y, dtype=np.float32)
